# revision 6
# baseline (speedup 1.0000x reference)
"""MetaNet (2-layer GNN) on 8 Trainium2 cores — v2.

Nodes are permuted by in-degree and dealt round-robin to cores; each core's
nodes are tiled 128 at a time with a uniform per-tile slot width S_t
(multiple of 4, >= tile max degree). scatter_mean becomes a windowed DVE
tensor_reduce over node-major slots; pad slots are zeroed by a -1e30 pad
indicator routed through the relu. The m-MLP second layer is applied per
node after aggregation. Layer 2 gathers AllGathered per-node projections
[Q2;R2] with one indirect DMA per 128-slot block.
"""

import os
import sys

sys.path.insert(0, "/opt/trn_rl_repo")

import numpy as np

import concourse.bass as bass
import concourse.mybir as mybir
import concourse.tile as tile
from concourse.bass_utils import run_bass_kernel_spmd

F32 = mybir.dt.float32
F32R = mybir.dt.float32r
BF16 = mybir.dt.bfloat16
I32 = mybir.dt.int32
P = 128
CW = 512
NCORES = 8
ACT = mybir.ActivationFunctionType
NEG = -1.0e30
SIM_NO_COLLECTIVE = False


def _r(ap):
    return ap.bitcast(F32R)


def _split_multi_waits(nc):
    n = 0
    for bb in nc.main_func.blocks:
        new_insts = []
        for ins in bb.instructions:
            si = getattr(ins, "sync_info", None)
            if si is not None and si.on_wait and len(si.on_wait) > 1:
                waits = list(si.on_wait)
                for w in waits[:-1]:
                    nop = mybir.InstNoOp(
                        name=f"wsplit_{n}",
                        engine=ins.engine,
                        bass_nofuse=True,
                        sync_info=mybir.SyncInfo(on_wait=[w], on_update=[]),
                    )
                    n += 1
                    new_insts.append(nop)
                si.on_wait = [waits[-1]]
            new_insts.append(ins)
        bb.instructions[:] = new_insts
    return n


def _host_prep(x, edge_attr, edge_index):
    N = x.shape[0]
    F = x.shape[1]
    FE = edge_attr.shape[1]
    npc = ((N + NCORES - 1) // NCORES + P - 1) // P * P
    NT = npc // P
    npad = npc * NCORES

    row = edge_index[0].astype(np.int64)
    col = edge_index[1].astype(np.int64)
    deg = np.bincount(row, minlength=npad).astype(np.int64)

    # permute nodes by degree, deal round-robin to cores
    rank_of = np.empty(npad, np.int64)
    rank_of[np.argsort(deg, kind="stable")] = np.arange(npad)
    core_of = rank_of % NCORES
    loc_of = rank_of // NCORES
    new_of = core_of * npc + loc_of       # old id -> packed new id
    old_of = np.empty(npad, np.int64)
    old_of[new_of] = np.arange(npad)

    # uniform S_t across cores: per tile max degree over all cores
    deg_new = deg[old_of]                 # by new id
    dt = deg_new.reshape(NCORES, NT, P)
    S_ALLOWED = [4, 8, 12, 16, 24, 32, 48]   # npc_ch = cw/S divides 128
    CW_OF = {4: 512, 8: 512, 12: 384, 16: 512, 24: 384, 32: 512, 48: 384}
    tmax = dt.max(axis=(0, 2))
    S_t = np.array([next(s for s in S_ALLOWED if s >= max(4, int(m)))
                    for m in tmax], np.int64)
    cw_t = np.array([CW_OF[int(s)] for s in S_t], np.int64)
    tile_slots = P * S_t
    tile_base = np.concatenate([[0], np.cumsum(tile_slots)])
    SLOT = int(tile_base[-1])
    G_t = S_t  # gather blocks per tile
    g_base = np.concatenate([[0], np.cumsum(G_t)])
    NGB = int(g_base[-1])

    # edge -> (core, slot)
    nd = new_of[row]
    c_e = nd // npc
    l_e = nd % npc
    t_e = l_e // P
    ln_e = l_e % P
    order = np.argsort(nd, kind="stable")
    k_e = np.empty(len(row), np.int64)
    nd_s = nd[order]
    first = np.r_[True, nd_s[1:] != nd_s[:-1]]
    idx_first = np.maximum.accumulate(np.where(first, np.arange(len(row)), 0))
    k_e[order] = np.arange(len(row)) - idx_first
    slot_e = tile_base[t_e] + ln_e * S_t[t_e] + k_e

    x_pad = np.zeros((npad, F), np.float32)
    x_pad[:N] = x

    # row order: [xcol(F), pi(1), xrow(F), eattr(FE)]
    ein1 = np.zeros((NCORES, 2 * F + FE + 1, SLOT), np.float32)
    ein1[:, F, :] = 1.0  # pad indicator, cleared for real slots
    colg = np.full((NCORES, SLOT), npc, np.int64)  # pad -> poison row
    ein1[c_e, 0:F, slot_e] = x_pad[col]
    ein1[c_e, F, slot_e] = 0.0
    ein1[c_e, F + 1:2 * F + 1, slot_e] = x[row]
    ein1[c_e, 2 * F + 1:, slot_e] = edge_attr
    gid_of = (new_of // npc) * (npc + 1) + (new_of % npc)
    colg[c_e, slot_e] = gid_of[col]

    # gather offsets: idx[c, p, g_base[t]+j] = colg[c, tile_base[t] + j*128 + p]
    idxg = np.zeros((NCORES, P, NGB), np.int32)
    for t in range(NT):
        blk = colg[:, tile_base[t]:tile_base[t + 1]].reshape(NCORES, int(G_t[t]), P)
        idxg[:, :, g_base[t]:g_base[t + 1]] = blk.transpose(0, 2, 1)

    cnt = deg_new.reshape(NCORES, npc)
    inv = np.where(cnt > 0, 1.0 / np.maximum(cnt, 1), 0.0).astype(np.float32)
    msk = (cnt > 0).astype(np.float32)

    xT = x_pad[old_of].reshape(NCORES, npc, F).transpose(0, 2, 1).copy()

    return dict(N=N, F=F, FE=FE, npc=npc, NT=NT, npad=npad, SLOT=SLOT,
                S_t=S_t, cw_t=cw_t, tile_base=tile_base, g_base=g_base, NGB=NGB,
                ein1=ein1, idxg=idxg, inv=inv, msk=msk, xT=xT,
                old_of=old_of)


def kernel(x, edge_attr, edge_index, **wts):
    x = np.asarray(x, np.float32)
    edge_attr = np.asarray(edge_attr, np.float32)
    edge_index = np.asarray(edge_index)
    wts = {k: np.asarray(v, np.float32) for k, v in wts.items()}
    return _run(x, edge_attr, edge_index, wts,
                trace=os.environ.get("BASS_KERNEL_TRACE", "0") == "1")


def _run(x, edge_attr, edge_index, wts, trace=False, build_only=False,
         sim_check=False):
    pre = _host_prep(x, edge_attr, edge_index)
    F, FE = pre["F"], pre["FE"]
    H = wts["e1_w2"].shape[1]
    npc, NT, npad, SLOT, NGB = (pre["npc"], pre["NT"], pre["npad"],
                                pre["SLOT"], pre["NGB"])
    S_t, tile_base, g_base = pre["S_t"], pre["tile_base"], pre["g_base"]
    cw_t = pre["cw_t"]

    # --- fold biases ---
    # l1 m bias: n1a_b1 + e1_b2 @ n1a_w1[ea side = rows F:]
    m1b1 = wts["n1a_b1"] + wts["e1_b2"] @ wts["n1a_w1"][F:]
    # l2 edge bias: e2_b1 + e1_b2 @ e2_w1[ea side = rows 2H:]
    e2b1 = wts["e2_b1"] + wts["e1_b2"] @ wts["e2_w1"][2 * H:]
    # l2 m bias: n2a_b1 + e2_b2 @ n2a_w1[ea side = rows H:]
    m2b1 = wts["n2a_b1"] + wts["e2_b2"] @ wts["n2a_w1"][H:]

    # l1 edge first layer: ein order [xrow, eattr, xcol, pi]
    e1w1 = np.concatenate([
        wts["e1_w1"][F:2 * F],        # xcol
        np.full((1, H), NEG, np.float32),  # pi -> h1 pad = 0 -> ea1 pad = 0
        wts["e1_w1"][0:F],            # xrow
        wts["e1_w1"][2 * F:2 * F + FE],  # eattr
    ])
    # l1 m first layer split: A (ea side, out 65), B (xcol+pi side, out 65)
    m1wA = np.zeros((H, H + 1), np.float32)
    m1wA[:, :H] = wts["n1a_w1"][F:]
    m1wB = np.zeros((F + 1, H + 1), np.float32)
    m1wB[:F, :H] = wts["n1a_w1"][:F]
    m1wB[F, :H] = NEG    # pi kills hm at pad slots
    m1wB[F, H] = NEG     # pi kills count indicator at pad slots
    m1b_aug = np.concatenate([m1b1, [1.0]]).reshape(H + 1, 1)
    m1w2aug = np.concatenate([wts["n1a_w2"], wts["n1a_b2"][None, :]])  # [65,64]

    # l1 node MLP
    n1w1x = wts["n1b_w1"][:F]        # rhs = xT
    n1w1a = wts["n1b_w1"][F:]        # rhs = hs
    # l2 QR projection: [Q2 | R2] = x1 @ [e2_w1 colside | n2a_w1 colside]
    wqr = np.concatenate([wts["e2_w1"][H:2 * H], wts["n2a_w1"][:H]], axis=1)
    qr_bias = np.concatenate([np.zeros(H, np.float32), m2b1]).reshape(2 * H, 1)
    # l2 P2 projection (row side)
    w_p2 = wts["e2_w1"][:H]
    m2w2 = wts["n2a_w2"]
    n2w1x = wts["n2b_w1"][:H]
    n2w1a = wts["n2b_w1"][H:]

    consts = dict(
        e1w1=e1w1, e1b1=wts["e1_b1"].reshape(H, 1), e1w2=wts["e1_w2"],
        m1wA=m1wA, m1wB=m1wB, m1b=m1b_aug, m1w2=m1w2aug,
        n1w1x=n1w1x, n1w1a=n1w1a, n1b1=wts["n1b_b1"].reshape(H, 1),
        n1w2=wts["n1b_w2"], n1b2=wts["n1b_b2"].reshape(H, 1),
        wqr=wqr, qrb=qr_bias, wp2=w_p2,
        e2w1e=wts["e2_w1"][2 * H:], e2b1=e2b1.reshape(H, 1),
        e2w2=wts["e2_w2"], m2a=wts["n2a_w1"][H:],
        m2w2=m2w2, m2b2=wts["n2a_b2"].reshape(1, H),
        n2w1x=n2w1x, n2w1a=n2w1a, n2b1=wts["n2b_b1"].reshape(H, 1),
        n2w2=wts["n2b_w2"],
        ident=np.eye(P, dtype=np.float32),
        zrow=np.concatenate([np.zeros((1, H), np.float32),
                             np.full((1, H), NEG, np.float32)], axis=1),
    )
    n2b2_val = float(wts["n2b_b2"].reshape(-1)[0])

    nc = bass.Bass(num_swdge_queues=4)

    W_KEYS = {"e1w1", "e1w2", "m1wA", "m1wB", "m1w2", "n1w1x", "n1w1a",
              "n1w2", "wqr", "wp2", "e2w1e", "e2w2", "m2a", "m2w2", "m2b2",
              "n2w1x", "n2w1a", "n2w2"}
    dp = {k: nc.declare_dram_parameter(k, list(v.shape),
                                       F32R if k in W_KEYS else F32,
                                       isOutput=False)
          for k, v in consts.items()}
    ein1_d = nc.declare_dram_parameter("ein1", [2 * F + FE + 1, SLOT], F32R,
                                       isOutput=False)
    idx_d = nc.declare_dram_parameter("idxg", [P, NGB], I32, isOutput=False)
    inv_d = nc.declare_dram_parameter("invrow", [1, npc], F32, isOutput=False)
    msk_d = nc.declare_dram_parameter("mskrow", [1, npc], F32R, isOutput=False)
    xT_d = nc.declare_dram_parameter("xT", [F, npc], F32R, isOutput=False)
    x2_d = nc.declare_dram_parameter("x2", [1, npc], F32, isOutput=True)

    with tile.TileContext(nc) as tc:
        with (
            tc.tile_pool(name="cst", bufs=1) as cst,
            tc.tile_pool(name="sbA", bufs=2) as sbA,
            tc.tile_pool(name="sbB", bufs=2) as sbB,
            tc.tile_pool(name="sbC", bufs=2) as sbC,
            tc.tile_pool(name="pA", bufs=2, space="PSUM") as pA,
            tc.tile_pool(name="pB", bufs=3, space="PSUM") as pB,
            tc.tile_pool(name="pE", bufs=2, space="PSUM") as pE,
            tc.tile_pool(name="pT", bufs=1, space="PSUM") as pT,
            tc.tile_pool(name="dram", bufs=1, space="DRAM") as dram,
        ):
            ct = {}
            for k, v in consts.items():
                dt_ = F32R if k in W_KEYS else F32
                t_ = cst.tile(list(v.shape), dt_, name=f"c_{k}")
                nc.sync.dma_start(out=t_[:], in_=dp[k][:])
                ct[k] = t_
            idx_sb = cst.tile([P, NGB], I32, name="c_idx")
            nc.sync.dma_start(out=idx_sb[:], in_=idx_d[:])

            ea1_d = dram.tile([H, SLOT], F32R, name="ea1")
            x1T_d = dram.tile([H, npc], F32R, name="x1T")
            qr_own_d = dram.tile([npc + 1, 2 * H], F32R, name="qrown")
            qr_full_d = dram.tile([(npc + 1) * NCORES, 2 * H], F32R,
                                  name="qrfull", addr_space="Shared")
            nc.sync.dma_start(out=qr_own_d[npc:npc + 1, :],
                              in_=_r(ct["zrow"][:]))

            SMAX = int(S_t.max())

            # ---------------- layer 1 ----------------
            for t in range(NT):
                S = int(S_t[t])
                cw = int(cw_t[t])
                nch = (S * P) // cw
                base = int(tile_base[t])
                npc_ch = cw // S  # nodes per chunk

                ein_t = sbA.tile([2 * F + FE + 1, SMAX * P], F32R, tag="ein")
                nc.sync.dma_start(out=ein_t[:, :S * P],
                                  in_=ein1_d[:, base:base + S * P])
                ea_t = sbA.tile([H, SMAX * P], F32R, tag="ea_t")
                agg = sbB.tile([H + 1, P], F32, tag="agg")
                invb = sbB.tile([H + 1, P], F32, tag="invb")
                nc.scalar.dma_start(
                    out=invb[:],
                    in_=inv_d[None, 0, t * P:(t + 1) * P]
                    .to_broadcast([H + 1, P]))
                xT_t = sbB.tile([F, P], F32R, tag="xT_t")
                nc.sync.dma_start(out=xT_t[:], in_=xT_d[:, t * P:(t + 1) * P])

                for c in range(nch):
                    lo = c * cw
                    h1_ps = pB.tile([H, CW], F32, tag="pB")
                    nc.tensor.matmul(h1_ps[:, :cw], lhsT=_r(ct["e1w1"][:]),
                                     rhs=ein_t[:, lo:lo + cw],
                                     start=True, stop=True)
                    h1r = sbC.tile([H, CW], F32, tag="h1r")
                    nc.scalar.activation(_r(h1r[:, :cw]), h1_ps[:, :cw],
                                         ACT.Relu, bias=ct["e1b1"][:, :1])
                    ea_ps = pE.tile([H, CW], F32, tag="pE")
                    nc.tensor.matmul(ea_ps[:, :cw], lhsT=_r(ct["e1w2"][:]),
                                     rhs=_r(h1r[:, :cw]), start=True, stop=True)
                    nc.scalar.activation(_r(ea_t[:, lo:lo + cw]),
                                         ea_ps[:, :cw], ACT.Copy)
                    hm_ps = pA.tile([H + 1, CW], F32, tag="pA")
                    nc.tensor.matmul(hm_ps[:, :cw], lhsT=_r(ct["m1wA"][:]),
                                     rhs=ea_t[:, lo:lo + cw],
                                     start=True, stop=False)
                    nc.tensor.matmul(hm_ps[:, :cw], lhsT=_r(ct["m1wB"][:]),
                                     rhs=ein_t[0:F + 1, lo:lo + cw],
                                     start=False, stop=True)
                    hm = sbC.tile([H + 1, CW], F32, tag="hm")
                    nc.scalar.activation(_r(hm[:, :cw]), hm_ps[:, :cw],
                                         ACT.Relu, bias=ct["m1b"][:, :1])
                    nc.vector.tensor_reduce(
                        out=agg[:, c * npc_ch:(c + 1) * npc_ch],
                        in_=hm[:, :cw].rearrange("h (n s) -> h n s", s=S),
                        axis=mybir.AxisListType.X, op=mybir.AluOpType.add)

                nc.sync.dma_start(out=ea1_d[:, base:base + S * P],
                                  in_=ea_t[:, :S * P])
                aggm = sbB.tile([H + 1, P], F32, tag="aggm")
                nc.vector.tensor_tensor(out=_r(aggm[:]), in0=agg[:],
                                        in1=invb[:], op=mybir.AluOpType.mult)
                hs_ps = pT.tile([P, P], F32, tag="pT")
                nc.tensor.matmul(hs_ps[0:H, :], lhsT=_r(ct["m1w2"][:]),
                                 rhs=_r(aggm[:]), start=True, stop=True)
                hs = sbB.tile([H, P], F32, tag="hs")
                nc.scalar.activation(_r(hs[:]), hs_ps[0:H, :], ACT.Copy)
                hn_ps = pT.tile([P, P], F32, tag="pT")
                nc.tensor.matmul(hn_ps[0:H, :], lhsT=_r(ct["n1w1a"][:]),
                                 rhs=_r(hs[:]), start=True, stop=False)
                nc.tensor.matmul(hn_ps[0:H, :], lhsT=_r(ct["n1w1x"][:]),
                                 rhs=xT_t[:], start=False, stop=True)
                hn = sbB.tile([H, P], F32, tag="hn")
                nc.scalar.activation(_r(hn[:]), hn_ps[0:H, :], ACT.Relu,
                                     bias=ct["n1b1"][:, :1])
                x1_ps = pT.tile([P, P], F32, tag="pT")
                nc.tensor.matmul(x1_ps[0:H, :], lhsT=_r(ct["n1w2"][:]),
                                 rhs=_r(hn[:]), start=True, stop=True)
                x1T_t = sbB.tile([H, P], F32, tag="x1T_t")
                nc.scalar.activation(_r(x1T_t[:]), x1_ps[0:H, :], ACT.Relu,
                                     bias=ct["n1b2"][:, :1])
                nc.sync.dma_start(out=x1T_d[:, t * P:(t + 1) * P],
                                  in_=_r(x1T_t[:]))
                qrT_ps = pT.tile([P, P], F32, tag="pT")
                nc.tensor.matmul(qrT_ps[:], lhsT=_r(ct["wqr"][:]),
                                 rhs=_r(x1T_t[:]), start=True, stop=True)
                qrT = sbB.tile([P, P], F32, tag="qrT")
                nc.vector.tensor_scalar(out=qrT[:], in0=qrT_ps[:],
                                        scalar1=ct["qrb"][:, :1], scalar2=None,
                                        op0=mybir.AluOpType.add)
                qr_ps = pT.tile([P, P], F32, tag="pT")
                nc.tensor.matmul(qr_ps[:], lhsT=qrT[:],
                                 rhs=ct["ident"][:].bitcast(F32),
                                 is_transpose=True, start=True, stop=True)
                qr_sb = sbB.tile([P, P], F32R, tag="qr_sb")
                nc.vector.tensor_copy(qr_sb[:], qr_ps[:])
                nc.sync.dma_start(out=qr_own_d[t * P:(t + 1) * P, :],
                                  in_=qr_sb[:])

            # ---------------- AllGather QR ----------------
            if SIM_NO_COLLECTIVE:
                qf = qr_full_d[:].rearrange("(c n) h -> c n h", c=NCORES)
                nc.sync.dma_start(
                    out=qf[:],
                    in_=qr_own_d[None, :, :].to_broadcast(
                        [NCORES, npc + 1, 2 * H]))
            else:
                nc.gpsimd.collective_compute(
                    "AllGather", mybir.AluOpType.bypass,
                    replica_groups=[list(range(NCORES))],
                    ins=[qr_own_d[:].opt()],
                    outs=[qr_full_d[:]
                          .rearrange("(c n) h -> c n h", c=NCORES).opt()])

            # ---------------- layer 2 ----------------
            for t in range(NT):
                S = int(S_t[t])
                cw = int(cw_t[t])
                nch = (S * P) // cw
                base = int(tile_base[t])
                gb = int(g_base[t])
                npc_ch = cw // S

                g_t = sbA.tile([P, SMAX * P], F32, tag="g_t")
                for j in range(S):
                    nc.gpsimd.indirect_dma_start(
                        out=g_t[:, j * P:(j + 1) * P],
                        out_offset=None,
                        in_=qr_full_d[:].bitcast(F32),
                        in_offset=bass.IndirectOffsetOnAxis(
                            ap=idx_sb[:, gb + j:gb + j + 1], axis=0))
                ea_t = sbA.tile([H, SMAX * P], F32R, tag="ea_t")
                nc.sync.dma_start(out=ea_t[:, :S * P],
                                  in_=ea1_d[:, base:base + S * P])
                agg = sbB.tile([H + 1, P], F32, tag="agg")
                invb = sbB.tile([H + 1, P], F32, tag="invb")
                nc.scalar.dma_start(
                    out=invb[:],
                    in_=inv_d[None, 0, t * P:(t + 1) * P]
                    .to_broadcast([H + 1, P]))
                mskr = sbB.tile([1, P], F32R, tag="mskr")
                nc.sync.dma_start(out=mskr[:],
                                  in_=msk_d[None, 0, t * P:(t + 1) * P])
                x1T_t = sbB.tile([H, P], F32R, tag="x1T_t2")
                nc.sync.dma_start(out=x1T_t[:], in_=x1T_d[:, t * P:(t + 1) * P])
                p2_ps = pT.tile([P, P], F32, tag="pT")
                nc.tensor.matmul(p2_ps[0:H, :], lhsT=_r(ct["wp2"][:]),
                                 rhs=x1T_t[:], start=True, stop=True)
                p2T = sbB.tile([H, P], F32, tag="p2T")
                nc.scalar.activation(p2T[:], p2_ps[0:H, :], ACT.Copy)

                for c in range(nch):
                    lo = c * cw
                    trA = pA.tile([P, CW], F32, tag="pA")
                    for k in range(cw // P):
                        nc.tensor.matmul(
                            trA[:, k * P:(k + 1) * P],
                            lhsT=g_t[:, lo + k * P:lo + (k + 1) * P],
                            rhs=ct["ident"][:].bitcast(F32),
                            is_transpose=True, start=True, stop=True)
                    hb_ps = pB.tile([H, CW], F32, tag="pB")
                    nc.tensor.matmul(hb_ps[:, :cw], lhsT=_r(ct["e2w1e"][:]),
                                     rhs=ea_t[:, lo:lo + cw],
                                     start=True, stop=True)
                    h1p = sbC.tile([H, CW], F32, tag="h1r")
                    nc.vector.tensor_tensor(
                        out=h1p[:, :cw], in0=hb_ps[:, :cw],
                        in1=p2T[:, c * npc_ch:(c + 1) * npc_ch]
                        .rearrange("h n -> h n ()")
                        .to_broadcast([H, npc_ch, S]),
                        op=mybir.AluOpType.add)
                    h1q = sbC.tile([H, CW], F32, tag="h1q")
                    nc.vector.tensor_tensor(out=h1q[:, :cw],
                                            in0=trA[0:H, :cw],
                                            in1=h1p[:, :cw],
                                            op=mybir.AluOpType.add)
                    h1r = sbC.tile([H, CW], F32, tag="h1r2")
                    nc.scalar.activation(_r(h1r[:, :cw]), h1q[:, :cw],
                                         ACT.Relu, bias=ct["e2b1"][:, :1])
                    ea2_ps = pE.tile([H, CW], F32, tag="pE")
                    nc.tensor.matmul(ea2_ps[:, :cw], lhsT=_r(ct["e2w2"][:]),
                                     rhs=_r(h1r[:, :cw]), start=True, stop=True)
                    ea2 = sbC.tile([H, CW], F32, tag="ea2")
                    nc.scalar.activation(_r(ea2[:, :cw]), ea2_ps[:, :cw],
                                         ACT.Copy)
                    hm_ps = pB.tile([H, CW], F32, tag="pB")
                    nc.tensor.matmul(hm_ps[:, :cw], lhsT=_r(ct["m2a"][:]),
                                     rhs=_r(ea2[:, :cw]), start=True, stop=True)
                    rT = sbC.tile([H, CW], F32, tag="rT")
                    nc.scalar.activation(rT[:, :cw], trA[H:2 * H, :cw],
                                         ACT.Copy)
                    hmp = sbC.tile([H, CW], F32, tag="hmp")
                    nc.vector.tensor_tensor(out=hmp[:, :cw],
                                            in0=hm_ps[:, :cw],
                                            in1=rT[:, :cw],
                                            op=mybir.AluOpType.add)
                    hm = sbC.tile([H, CW], F32, tag="hm2")
                    nc.scalar.activation(_r(hm[:, :cw]), hmp[:, :cw], ACT.Relu)
                    nc.vector.tensor_reduce(
                        out=agg[0:H, c * npc_ch:(c + 1) * npc_ch],
                        in_=hm[:, :cw].rearrange("h (n s) -> h n s", s=S),
                        axis=mybir.AxisListType.X, op=mybir.AluOpType.add)

                aggm = sbB.tile([H + 1, P], F32, tag="aggm")
                nc.vector.tensor_tensor(out=_r(aggm[0:H, :]), in0=agg[0:H, :],
                                        in1=invb[0:H, :],
                                        op=mybir.AluOpType.mult)
                hs_ps = pT.tile([P, P], F32, tag="pT")
                nc.tensor.matmul(hs_ps[0:H, :], lhsT=_r(ct["m2w2"][:]),
                                 rhs=_r(aggm[0:H, :]), start=True, stop=False)
                nc.tensor.matmul(hs_ps[0:H, :], lhsT=_r(ct["m2b2"][:]),
                                 rhs=mskr[:], start=False, stop=True)
                hs = sbB.tile([H, P], F32, tag="hs")
                nc.scalar.activation(_r(hs[:]), hs_ps[0:H, :], ACT.Copy)
                hn_ps = pT.tile([P, P], F32, tag="pT")
                nc.tensor.matmul(hn_ps[0:H, :], lhsT=_r(ct["n2w1a"][:]),
                                 rhs=_r(hs[:]), start=True, stop=False)
                nc.tensor.matmul(hn_ps[0:H, :], lhsT=_r(ct["n2w1x"][:]),
                                 rhs=x1T_t[:], start=False, stop=True)
                hn = sbB.tile([H, P], F32, tag="hn")
                nc.scalar.activation(_r(hn[:]), hn_ps[0:H, :], ACT.Relu,
                                     bias=ct["n2b1"][:, :1])
                x2_ps = pT.tile([P, P], F32, tag="pT")
                nc.tensor.matmul(x2_ps[0:1, :], lhsT=_r(ct["n2w2"][:]),
                                 rhs=_r(hn[:]), start=True, stop=True)
                x2sb = sbB.tile([1, P], F32, tag="x2sb")
                nc.scalar.activation(x2sb[:], x2_ps[0:1, :], ACT.Copy,
                                     bias=n2b2_val)
                nc.gpsimd.dma_start(out=x2_d[:, t * P:(t + 1) * P],
                                    in_=x2sb[:])

    if not sim_check:
        _split_multi_waits(nc)

    in_maps = []
    for c in range(NCORES):
        m = dict(consts)
        m["ein1"] = pre["ein1"][c]
        m["idxg"] = pre["idxg"][c]
        m["invrow"] = pre["inv"][c].reshape(1, npc)
        m["mskrow"] = pre["msk"][c].reshape(1, npc)
        m["xT"] = pre["xT"][c]
        in_maps.append(m)

    kernel.last_nc = nc
    kernel.last_in_maps = in_maps
    kernel.last_pre = pre
    if build_only:
        return pre
    if sim_check:
        from concourse.bass_interp import MultiCoreSim
        sim = MultiCoreSim(nc, num_cores=NCORES, require_finite=False,
                           require_nnan=False)
        for ci, core in sim.cores.items():
            for n, v in in_maps[ci].items():
                core.tensor(n)[:] = v
        sim.simulate(check_with_hw=False)
        outs = [np.array(sim.cores[ci].tensor("x2")) for ci in range(NCORES)]
        full = np.concatenate([o.reshape(-1) for o in outs])
    else:
        r = run_bass_kernel_spmd(nc, in_maps, list(range(NCORES)), trace=trace)
        kernel.last_results = r
        full = np.concatenate(
            [r.results[c]["x2"].reshape(-1) for c in range(NCORES)])
    out = np.empty((npad, 1), np.float32)
    out[pre["old_of"], 0] = full  # wait: full is by new id; old_of[new]=old
    return out[:pre["N"]].astype(np.float32)
